# revision 3
# baseline (speedup 1.0000x reference)
"""CAML attention kernel for Trainium2 (8 NeuronCores, SPMD over classes).

Reference computation:
    xt      = tanh(x)                      # [B, D, L]
    scores  = einsum('cd,bdl->bcl', W1, xt)
    weights = softmax(scores, axis=l)
    weighted= einsum('bcl,bdl->bcd', weights, xt)
    out     = einsum('cd,bcd->bc', W2, weighted) + b2

Key identity used here: the final contraction commutes with the softmax
weighted sum, so with s2 = einsum('cd,bdl->bcl', W2, xt):
    out[b,c] = sum_l softmax(s1[b,c,:])[l] * s2[b,c,l] + b2[c]
             = (sum_l exp(s1)*s2) / (sum_l exp(s1)) + b2
(|s1| <= 512*max|W1| ~ 13, so exp without max-subtraction is safe in fp32.)

This removes the [B,C,D] intermediate and the L-on-partition transpose that a
direct implementation of the second einsum would need: both big matmuls have
the same (contract over D) orientation, softmax + weighting reduce along the
free axis, fused into one ACT op (exp + accumulated denominator) and one DVE
op (tensor_tensor_reduce: product + accumulated numerator).

Sharding: C padded 8930 -> 9216 = 8 cores * 1152; weights row-sharded per
core, x replicated. Zero-padded weight rows give out=0 there (exp(0) rows
reduce to 0/denom + 0), discarded on the host after gathering.
"""

import numpy as np
import ml_dtypes

import concourse.bacc as bacc
import concourse.tile as tile
from concourse import mybir
from concourse.bass import ts
from concourse.bass_utils import run_bass_kernel_spmd

B, D, L, C = 8, 512, 2500, 8930
N_CORES = 8
P = 128

C_PAD = 9216                 # next multiple of 8*128 above C
C_SH = C_PAD // N_CORES      # 1152 classes per core
KCH = D // P                 # 4 contraction chunks
JCH = C_SH // P              # 9 class chunks per core
LCH = 5                      # l chunks
LT = L // LCH                # 500 columns per matmul (fits one PSUM bank)

F32 = mybir.dt.float32
BF16 = mybir.dt.bfloat16


def build_nc(b=B, kch=KCH, jch=JCH, lch=LCH, lt=LT):
    """Emit the per-core program. All cores run the same NEFF (SPMD)."""
    nc = bacc.Bacc("TRN2", target_bir_lowering=False, debug=False)

    x = nc.dram_tensor("x", [b, kch, P, lch * lt], F32, kind="ExternalInput")
    w1t = nc.dram_tensor("w1t", [kch, P, jch * P], BF16, kind="ExternalInput")
    w2t = nc.dram_tensor("w2t", [kch, P, jch * P], BF16, kind="ExternalInput")
    b2s = nc.dram_tensor("b2s", [P, jch], F32, kind="ExternalInput")
    out = nc.dram_tensor("out", [jch, P, b], F32, kind="ExternalOutput")

    Exp = mybir.ActivationFunctionType.Exp
    Tanh = mybir.ActivationFunctionType.Tanh
    mult = mybir.AluOpType.mult
    add = mybir.AluOpType.add
    AX = mybir.AxisListType.X

    with tile.TileContext(nc) as tc:
        with (
            tc.tile_pool(name="wts", bufs=1) as wpool,
            tc.tile_pool(name="xraw", bufs=3) as xpool,
            tc.tile_pool(name="xt", bufs=2 * kch) as xtpool,
            tc.tile_pool(name="ps1", bufs=2, space="PSUM") as ppool1,
            tc.tile_pool(name="ps2", bufs=2, space="PSUM") as ppool2,
            tc.tile_pool(name="etile", bufs=3) as epool,
            tc.tile_pool(name="scratch", bufs=2) as spool,
            tc.tile_pool(name="cols", bufs=3) as cpool,
            tc.tile_pool(name="outp", bufs=1) as opool,
        ):
            w1sb = wpool.tile([P, kch, jch * P], BF16)
            w2sb = wpool.tile([P, kch, jch * P], BF16)
            b2sb = wpool.tile([P, jch], F32)
            for k in range(kch):
                nc.sync.dma_start(out=w1sb[:, k], in_=w1t[k])
                nc.sync.dma_start(out=w2sb[:, k], in_=w2t[k])
            nc.sync.dma_start(out=b2sb, in_=b2s[:])

            out_all = opool.tile([P, jch, b], F32)

            for bi in range(b):
                xts = []
                for k in range(kch):
                    xraw = xpool.tile([P, lch * lt], F32)
                    nc.sync.dma_start(out=xraw, in_=x[bi, k])
                    xt_k = xtpool.tile([P, lch * lt], BF16, tag="xt")
                    nc.scalar.activation(out=xt_k, in_=xraw, func=Tanh)
                    xts.append(xt_k)

                for j in range(jch):
                    denom_cols = cpool.tile([P, lch], F32, tag="dcols")
                    numer_cols = cpool.tile([P, lch], F32, tag="ncols")
                    for l in range(lch):
                        s1 = ppool1.tile([P, lt], F32)
                        s2 = ppool2.tile([P, lt], F32)
                        for k in range(kch):
                            nc.tensor.matmul(
                                s1,
                                w1sb[:, k, ts(j, P)],
                                xts[k][:, ts(l, lt)],
                                start=(k == 0),
                                stop=(k == kch - 1),
                            )
                        for k in range(kch):
                            nc.tensor.matmul(
                                s2,
                                w2sb[:, k, ts(j, P)],
                                xts[k][:, ts(l, lt)],
                                start=(k == 0),
                                stop=(k == kch - 1),
                            )
                        e = epool.tile([P, lt], F32)
                        nc.scalar.activation(
                            out=e, in_=s1, func=Exp,
                            accum_out=denom_cols[:, l : l + 1],
                        )
                        prod = spool.tile([P, lt], F32)
                        # numer partial = sum_l E * s2 (tensor_tensor_reduce
                        # doesn't execute on this runtime; STT with accum_out
                        # is the same single DVE pass)
                        nc.vector.scalar_tensor_tensor(
                            out=prod, in0=e, scalar=1.0, in1=s2,
                            op0=mult, op1=mult,
                            accum_out=numer_cols[:, l : l + 1],
                        )
                    denom = cpool.tile([P, 1], F32, tag="dsum")
                    numer = cpool.tile([P, 1], F32, tag="nsum")
                    recip = cpool.tile([P, 1], F32, tag="rsum")
                    nc.vector.reduce_sum(denom, denom_cols, axis=AX)
                    nc.vector.reduce_sum(numer, numer_cols, axis=AX)
                    nc.vector.reciprocal(recip, denom)
                    # out = numer * (1/denom) + b2
                    nc.vector.scalar_tensor_tensor(
                        out=out_all[:, j, bi : bi + 1],
                        in0=numer, scalar=recip, in1=b2sb[:, j : j + 1],
                        op0=mult, op1=add,
                    )

            for j in range(jch):
                nc.sync.dma_start(out=out[j], in_=out_all[:, j])

    nc.compile()
    return nc


_NC_CACHE = {}


def _get_nc():
    if "nc" not in _NC_CACHE:
        _NC_CACHE["nc"] = build_nc()
    return _NC_CACHE["nc"]


def make_in_maps(x, W1, W2, b2):
    """Host-side shard prep: pad C, pre-transpose weights, cast to bf16."""
    x = np.ascontiguousarray(np.asarray(x, dtype=np.float32)).reshape(B, KCH, P, L)

    def prep_w(W):
        Wp = np.zeros((C_PAD, D), dtype=np.float32)
        Wp[:C] = np.asarray(W, dtype=np.float32)
        return Wp

    W1p, W2p = prep_w(W1), prep_w(W2)
    b2p = np.zeros((C_PAD,), dtype=np.float32)
    b2p[:C] = np.asarray(b2, dtype=np.float32)

    in_maps = []
    for i in range(N_CORES):
        sl = slice(i * C_SH, (i + 1) * C_SH)
        w1t = np.ascontiguousarray(W1p[sl].T).reshape(KCH, P, C_SH)
        w2t = np.ascontiguousarray(W2p[sl].T).reshape(KCH, P, C_SH)
        b2s = np.ascontiguousarray(b2p[sl].reshape(JCH, P).T)
        in_maps.append(
            {
                "x": x,
                "w1t": w1t.astype(ml_dtypes.bfloat16),
                "w2t": w2t.astype(ml_dtypes.bfloat16),
                "b2s": b2s,
            }
        )
    return in_maps


def gather_out(results):
    """results: list (per core) of {'out': [JCH, P, B]} -> full [B, C]."""
    parts = [
        np.transpose(np.asarray(r["out"], dtype=np.float32), (2, 0, 1)).reshape(B, C_SH)
        for r in results
    ]
    return np.concatenate(parts, axis=1)[:, :C]


def kernel(x, W1, W2, b2):
    nc = _get_nc()
    in_maps = make_in_maps(x, W1, W2, b2)
    res = run_bass_kernel_spmd(nc, in_maps, list(range(N_CORES)))
    return gather_out(res.results)


# revision 6
# speedup vs baseline: 1.0203x; 1.0203x over previous
"""CAML attention kernel for Trainium2 (8 NeuronCores, SPMD over classes).

Reference computation:
    xt      = tanh(x)                      # [B, D, L]
    scores  = einsum('cd,bdl->bcl', W1, xt)
    weights = softmax(scores, axis=l)
    weighted= einsum('bcl,bdl->bcd', weights, xt)
    out     = einsum('cd,bcd->bc', W2, weighted) + b2

Key identity used here: the final contraction commutes with the softmax
weighted sum, so with s2 = einsum('cd,bdl->bcl', W2, xt):
    out[b,c] = sum_l softmax(s1[b,c,:])[l] * s2[b,c,l] + b2[c]
             = (sum_l exp(s1)*s2) / (sum_l exp(s1)) + b2
(|s1| <= 512*max|W1| ~ 13, so exp without max-subtraction is safe in fp32.)

This removes the [B,C,D] intermediate and the L-on-partition transpose that a
direct implementation of the second einsum would need: both big matmuls have
the same (contract over D) orientation, softmax + weighting reduce along the
free axis, fused into one ACT op (exp + accumulated denominator) and one DVE
op (tensor_tensor_reduce: product + accumulated numerator).

Sharding: C padded 8930 -> 9216 = 8 cores * 1152; weights row-sharded per
core, x replicated. Zero-padded weight rows give out=0 there (exp(0) rows
reduce to 0/denom + 0), discarded on the host after gathering.
"""

import numpy as np
import ml_dtypes

import concourse.bacc as bacc
import concourse.tile as tile
from concourse import mybir
from concourse.bass import ts
from concourse.bass_utils import run_bass_kernel_spmd

B, D, L, C = 8, 512, 2500, 8930
N_CORES = 8
P = 128

C_PAD = 9216                 # next multiple of 8*128 above C
C_SH = C_PAD // N_CORES      # 1152 classes per core
KCH = D // P                 # 4 contraction chunks
JCH = C_SH // P              # 9 class chunks per core
LCH = 5                      # l chunks
LT = L // LCH                # 500 columns per matmul (fits one PSUM bank)

F32 = mybir.dt.float32
BF16 = mybir.dt.bfloat16


def build_nc(b=B, kch=KCH, jch=JCH, lch=LCH, lt=LT):
    """Emit the per-core program. All cores run the same NEFF (SPMD)."""
    nc = bacc.Bacc("TRN2", target_bir_lowering=False, debug=False)

    x = nc.dram_tensor("x", [b, kch, P, lch * lt], F32, kind="ExternalInput")
    w1t = nc.dram_tensor("w1t", [kch, P, jch * P], BF16, kind="ExternalInput")
    w2t = nc.dram_tensor("w2t", [kch, P, jch * P], BF16, kind="ExternalInput")
    b2s = nc.dram_tensor("b2s", [P, jch], F32, kind="ExternalInput")
    out = nc.dram_tensor("out", [jch, P, b], F32, kind="ExternalOutput")

    Exp = mybir.ActivationFunctionType.Exp
    Tanh = mybir.ActivationFunctionType.Tanh
    mult = mybir.AluOpType.mult
    add = mybir.AluOpType.add
    AX = mybir.AxisListType.X

    with tile.TileContext(nc) as tc:
        with (
            tc.tile_pool(name="wts", bufs=1) as wpool,
            tc.tile_pool(name="xraw", bufs=8) as xpool,
            tc.tile_pool(name="xt", bufs=2 * kch * lch) as xtpool,
            tc.tile_pool(name="ps1", bufs=3, space="PSUM") as ppool1,
            tc.tile_pool(name="ps2", bufs=3, space="PSUM") as ppool2,
            tc.tile_pool(name="etile", bufs=3) as epool,
            tc.tile_pool(name="scratch", bufs=2) as spool,
            tc.tile_pool(name="cols", bufs=3) as cpool,
            tc.tile_pool(name="outp", bufs=1) as opool,
        ):
            # weights go on the gpsimd DMA queue so they don't serialize
            # ahead of the x tiles on the sync queue
            w1sb = wpool.tile([P, kch, jch * P], BF16)
            w2sb = wpool.tile([P, kch, jch * P], BF16)
            b2sb = wpool.tile([P, jch], F32)
            for k in range(kch):
                nc.gpsimd.dma_start(out=w1sb[:, k], in_=w1t[k])
                nc.gpsimd.dma_start(out=w2sb[:, k], in_=w2t[k])
            nc.gpsimd.dma_start(out=b2sb, in_=b2s[:])

            out_all = opool.tile([P, jch, b], F32)

            for bi in range(b):
                # load + tanh at (k, l-chunk) granularity, l-major order, so
                # the first matmul group's inputs land as early as possible
                xts = {}
                for l in range(lch):
                    for k in range(kch):
                        xraw = xpool.tile([P, lt], F32)
                        nc.sync.dma_start(
                            out=xraw, in_=x[bi, k, :, l * lt : (l + 1) * lt]
                        )
                        xt_kl = xtpool.tile([P, lt], BF16, tag="xt")
                        nc.scalar.activation(out=xt_kl, in_=xraw, func=Tanh)
                        xts[(k, l)] = xt_kl

                for j in range(jch):
                    denom_cols = cpool.tile([P, lch], F32, tag="dcols")
                    numer_cols = cpool.tile([P, lch], F32, tag="ncols")
                    for l in range(lch):
                        s1 = ppool1.tile([P, lt], F32)
                        s2 = ppool2.tile([P, lt], F32)
                        for k in range(kch):
                            nc.tensor.matmul(
                                s1,
                                w1sb[:, k, ts(j, P)],
                                xts[(k, l)],
                                start=(k == 0),
                                stop=(k == kch - 1),
                            )
                        for k in range(kch):
                            nc.tensor.matmul(
                                s2,
                                w2sb[:, k, ts(j, P)],
                                xts[(k, l)],
                                start=(k == 0),
                                stop=(k == kch - 1),
                            )
                        e = epool.tile([P, lt], F32)
                        nc.scalar.activation(
                            out=e, in_=s1, func=Exp,
                            accum_out=denom_cols[:, l : l + 1],
                        )
                        prod = spool.tile([P, lt], F32)
                        # numer partial = sum_l E * s2 (tensor_tensor_reduce
                        # doesn't execute on this runtime; STT with accum_out
                        # is the same single DVE pass)
                        nc.vector.scalar_tensor_tensor(
                            out=prod, in0=e, scalar=1.0, in1=s2,
                            op0=mult, op1=mult,
                            accum_out=numer_cols[:, l : l + 1],
                        )
                    denom = cpool.tile([P, 1], F32, tag="dsum")
                    numer = cpool.tile([P, 1], F32, tag="nsum")
                    recip = cpool.tile([P, 1], F32, tag="rsum")
                    nc.vector.reduce_sum(denom, denom_cols, axis=AX)
                    nc.vector.reduce_sum(numer, numer_cols, axis=AX)
                    nc.vector.reciprocal(recip, denom)
                    # out = numer * (1/denom) + b2
                    nc.vector.scalar_tensor_tensor(
                        out=out_all[:, j, bi : bi + 1],
                        in0=numer, scalar=recip, in1=b2sb[:, j : j + 1],
                        op0=mult, op1=add,
                    )
                    if bi == b - 1:
                        nc.sync.dma_start(out=out[j], in_=out_all[:, j])

    nc.compile()
    return nc


_NC_CACHE = {}


def _get_nc():
    if "nc" not in _NC_CACHE:
        _NC_CACHE["nc"] = build_nc()
    return _NC_CACHE["nc"]


def make_in_maps(x, W1, W2, b2):
    """Host-side shard prep: pad C, pre-transpose weights, cast to bf16."""
    x = np.ascontiguousarray(np.asarray(x, dtype=np.float32)).reshape(B, KCH, P, L)

    def prep_w(W):
        Wp = np.zeros((C_PAD, D), dtype=np.float32)
        Wp[:C] = np.asarray(W, dtype=np.float32)
        return Wp

    W1p, W2p = prep_w(W1), prep_w(W2)
    b2p = np.zeros((C_PAD,), dtype=np.float32)
    b2p[:C] = np.asarray(b2, dtype=np.float32)

    in_maps = []
    for i in range(N_CORES):
        sl = slice(i * C_SH, (i + 1) * C_SH)
        w1t = np.ascontiguousarray(W1p[sl].T).reshape(KCH, P, C_SH)
        w2t = np.ascontiguousarray(W2p[sl].T).reshape(KCH, P, C_SH)
        b2s = np.ascontiguousarray(b2p[sl].reshape(JCH, P).T)
        in_maps.append(
            {
                "x": x,
                "w1t": w1t.astype(ml_dtypes.bfloat16),
                "w2t": w2t.astype(ml_dtypes.bfloat16),
                "b2s": b2s,
            }
        )
    return in_maps


def gather_out(results):
    """results: list (per core) of {'out': [JCH, P, B]} -> full [B, C]."""
    parts = [
        np.transpose(np.asarray(r["out"], dtype=np.float32), (2, 0, 1)).reshape(B, C_SH)
        for r in results
    ]
    return np.concatenate(parts, axis=1)[:, :C]


def kernel(x, W1, W2, b2):
    nc = _get_nc()
    in_maps = make_in_maps(x, W1, W2, b2)
    res = run_bass_kernel_spmd(nc, in_maps, list(range(N_CORES)))
    return gather_out(res.results)


# revision 14
# speedup vs baseline: 1.0253x; 1.0049x over previous
"""CAML attention kernel for Trainium2 (8 NeuronCores, SPMD over classes).

Reference computation:
    xt      = tanh(x)                      # [B, D, L]
    scores  = einsum('cd,bdl->bcl', W1, xt)
    weights = softmax(scores, axis=l)
    weighted= einsum('bcl,bdl->bcd', weights, xt)
    out     = einsum('cd,bcd->bc', W2, weighted) + b2

Key identity used here: the final contraction commutes with the softmax
weighted sum, so with s2 = einsum('cd,bdl->bcl', W2, xt):
    out[b,c] = sum_l softmax(s1[b,c,:])[l] * s2[b,c,l] + b2[c]
             = (sum_l exp(s1)*s2) / (sum_l exp(s1)) + b2
(|s1| <= 512*max|W1| ~ 13, so exp without max-subtraction is safe in fp32.)

This removes the [B,C,D] intermediate and the L-on-partition transpose that a
direct implementation of the second einsum would need: both big matmuls have
the same (contract over D) orientation, softmax + weighting reduce along the
free axis, fused into one ACT op (exp + accumulated denominator) and one DVE
op (scalar_tensor_tensor: product + accumulated numerator).

Sharding: C padded 8930 -> 9216 = 8 cores * 1152; weights row-sharded per
core, x replicated. Zero-padded weight rows give out=0 there (exp(0) rows
reduce to 0/denom + 0), discarded on the host after gathering.
"""

import numpy as np
import ml_dtypes

import concourse.bacc as bacc
import concourse.tile as tile
from concourse import mybir
from concourse.bass import ts
from concourse.bass_utils import run_bass_kernel_spmd

B, D, L, C = 8, 512, 2500, 8930
N_CORES = 8
P = 128

C_PAD = 9216                 # next multiple of 8*128 above C
C_SH = C_PAD // N_CORES      # 1152 classes per core
KCH = D // P                 # 4 contraction chunks
JCH = C_SH // P              # 9 class chunks per core
LCH = 5                      # l chunks
LT = L // LCH                # 500 columns per matmul (fits one PSUM bank)

F32 = mybir.dt.float32
BF16 = mybir.dt.bfloat16


def build_nc(b=B, kch=KCH, jch=JCH, lch=LCH, lt=LT):
    """Emit the per-core program. All cores run the same NEFF (SPMD)."""
    nc = bacc.Bacc("TRN2", target_bir_lowering=False, debug=False)

    x = nc.dram_tensor("x", [b, kch, P, lch * lt], F32, kind="ExternalInput")
    w1t = nc.dram_tensor("w1t", [kch, P, jch * P], BF16, kind="ExternalInput")
    w2t = nc.dram_tensor("w2t", [kch, P, jch * P], BF16, kind="ExternalInput")
    b2s = nc.dram_tensor("b2s", [P, jch], F32, kind="ExternalInput")
    out = nc.dram_tensor("out", [jch, P, b], F32, kind="ExternalOutput")

    Exp = mybir.ActivationFunctionType.Exp
    Tanh = mybir.ActivationFunctionType.Tanh
    mult = mybir.AluOpType.mult
    add = mybir.AluOpType.add
    AX = mybir.AxisListType.X

    with tile.TileContext(nc) as tc:
        with (
            tc.tile_pool(name="wts", bufs=1) as wpool,
            tc.tile_pool(name="xraw", bufs=8) as xpool,
            tc.tile_pool(name="xt", bufs=2 * kch * lch) as xtpool,
            tc.tile_pool(name="ps1", bufs=4, space="PSUM") as ppool1,
            tc.tile_pool(name="ps2", bufs=4, space="PSUM") as ppool2,
            tc.tile_pool(name="etile", bufs=6) as epool,
            tc.tile_pool(name="scratch", bufs=4) as spool,
            tc.tile_pool(name="cols", bufs=6) as cpool,
            tc.tile_pool(name="outp", bufs=1) as opool,
        ):
            # one fast HWDGE queue, ordered by first consumption: the first
            # matmul group (j=0, l=0 of batch 0) needs w1 + the four l=0
            # x chunks, then w2 for its s2 half; everything else follows
            w1sb = wpool.tile([P, kch, jch * P], BF16)
            w2sb = wpool.tile([P, kch, jch * P], BF16)
            b2sb = wpool.tile([P, jch], F32)
            for k in range(kch):
                nc.sync.dma_start(out=w1sb[:, k], in_=w1t[k])

            out_all = opool.tile([P, jch, b], F32)

            for bi in range(b):
                # load + tanh at (k, l-chunk) granularity, l-major order, so
                # the first matmul group's inputs land as early as possible
                xts = {}
                for l in range(lch):
                    for k in range(kch):
                        xraw = xpool.tile([P, lt], F32)
                        nc.sync.dma_start(
                            out=xraw, in_=x[bi, k, :, l * lt : (l + 1) * lt]
                        )
                        xt_kl = xtpool.tile([P, lt], BF16, tag="xt")
                        nc.scalar.activation(out=xt_kl, in_=xraw, func=Tanh)
                        xts[(k, l)] = xt_kl
                    if bi == 0 and l == 0:
                        for k in range(kch):
                            nc.sync.dma_start(out=w2sb[:, k], in_=w2t[k])
                        nc.sync.dma_start(out=b2sb, in_=b2s[:])

                for j in range(jch):
                    denom_cols = cpool.tile([P, lch], F32, tag="dcols")
                    numer_cols = cpool.tile([P, lch], F32, tag="ncols")
                    for l in range(lch):
                        s1 = ppool1.tile([P, lt], F32)
                        s2 = ppool2.tile([P, lt], F32)
                        for k in range(kch):
                            nc.tensor.matmul(
                                s1,
                                w1sb[:, k, ts(j, P)],
                                xts[(k, l)],
                                start=(k == 0),
                                stop=(k == kch - 1),
                            )
                        for k in range(kch):
                            nc.tensor.matmul(
                                s2,
                                w2sb[:, k, ts(j, P)],
                                xts[(k, l)],
                                start=(k == 0),
                                stop=(k == kch - 1),
                            )
                        e = epool.tile([P, lt], F32)
                        nc.scalar.activation(
                            out=e, in_=s1, func=Exp,
                            accum_out=denom_cols[:, l : l + 1],
                        )
                        prod = spool.tile([P, lt], F32)
                        # numer partial = sum_l E * s2 (tensor_tensor_reduce
                        # doesn't execute on this runtime; STT with accum_out
                        # is the same single DVE pass)
                        nc.vector.scalar_tensor_tensor(
                            out=prod, in0=e, scalar=1.0, in1=s2,
                            op0=mult, op1=mult,
                            accum_out=numer_cols[:, l : l + 1],
                        )
                    denom = cpool.tile([P, 1], F32, tag="dsum")
                    numer = cpool.tile([P, 1], F32, tag="nsum")
                    recip = cpool.tile([P, 1], F32, tag="rsum")
                    nc.vector.reduce_sum(denom, denom_cols, axis=AX)
                    nc.vector.reduce_sum(numer, numer_cols, axis=AX)
                    nc.vector.reciprocal(recip, denom)
                    # out = numer * (1/denom) + b2
                    nc.vector.scalar_tensor_tensor(
                        out=out_all[:, j, bi : bi + 1],
                        in0=numer, scalar=recip, in1=b2sb[:, j : j + 1],
                        op0=mult, op1=add,
                    )
                    if bi == b - 1:
                        nc.sync.dma_start(out=out[j], in_=out_all[:, j])

    nc.compile()
    return nc


_NC_CACHE = {}


def _get_nc():
    if "nc" not in _NC_CACHE:
        _NC_CACHE["nc"] = build_nc()
    return _NC_CACHE["nc"]


def make_in_maps(x, W1, W2, b2):
    """Host-side shard prep: pad C, pre-transpose weights, cast to bf16."""
    x = np.ascontiguousarray(np.asarray(x, dtype=np.float32)).reshape(B, KCH, P, L)

    def prep_w(W):
        Wp = np.zeros((C_PAD, D), dtype=np.float32)
        Wp[:C] = np.asarray(W, dtype=np.float32)
        return Wp

    W1p, W2p = prep_w(W1), prep_w(W2)
    b2p = np.zeros((C_PAD,), dtype=np.float32)
    b2p[:C] = np.asarray(b2, dtype=np.float32)

    in_maps = []
    for i in range(N_CORES):
        sl = slice(i * C_SH, (i + 1) * C_SH)
        w1t = np.ascontiguousarray(W1p[sl].T).reshape(KCH, P, C_SH)
        w2t = np.ascontiguousarray(W2p[sl].T).reshape(KCH, P, C_SH)
        b2s = np.ascontiguousarray(b2p[sl].reshape(JCH, P).T)
        in_maps.append(
            {
                "x": x,
                "w1t": w1t.astype(ml_dtypes.bfloat16),
                "w2t": w2t.astype(ml_dtypes.bfloat16),
                "b2s": b2s,
            }
        )
    return in_maps


def gather_out(results):
    """results: list (per core) of {'out': [JCH, P, B]} -> full [B, C]."""
    parts = [
        np.transpose(np.asarray(r["out"], dtype=np.float32), (2, 0, 1)).reshape(B, C_SH)
        for r in results
    ]
    return np.concatenate(parts, axis=1)[:, :C]


def kernel(x, W1, W2, b2):
    nc = _get_nc()
    in_maps = make_in_maps(x, W1, W2, b2)
    res = run_bass_kernel_spmd(nc, in_maps, list(range(N_CORES)))
    return gather_out(res.results)


# revision 22
# speedup vs baseline: 1.3237x; 1.2910x over previous
"""CAML attention kernel for Trainium2 (8 NeuronCores, SPMD over classes).

Reference computation:
    xt      = tanh(x)                      # [B, D, L]
    scores  = einsum('cd,bdl->bcl', W1, xt)
    weights = softmax(scores, axis=l)
    weighted= einsum('bcl,bdl->bcd', weights, xt)
    out     = einsum('cd,bcd->bc', W2, weighted) + b2

Key identity used here: the final contraction commutes with the softmax
weighted sum, so with s2 = einsum('cd,bdl->bcl', W2, xt):
    out[b,c] = sum_l softmax(s1[b,c,:])[l] * s2[b,c,l] + b2[c]
             = (sum_l exp(s1)*s2) / (sum_l exp(s1)) + b2
(|s1| <= 512*max|W1| ~ 13, so exp without max-subtraction is safe in fp32.)

This removes the [B,C,D] intermediate and the L-on-partition transpose that a
direct implementation of the second einsum would need: both big matmuls have
the same (contract over D) orientation, softmax + weighting reduce along the
free axis, fused into one ACT op (exp + accumulated denominator) and one DVE
op (scalar_tensor_tensor: product + accumulated numerator).

Sharding: C padded 8930 -> 9216 = 8 cores * 1152; weights row-sharded per
core, x replicated. Zero-padded weight rows give out=0 there (exp(0) rows
reduce to 0/denom + 0), discarded on the host after gathering.
"""

import numpy as np
import ml_dtypes

import concourse.bacc as bacc
import concourse.tile as tile
from concourse import mybir
from concourse.bass import ts
from concourse.bass_utils import run_bass_kernel_spmd

B, D, L, C = 8, 512, 2500, 8930
N_CORES = 8
P = 128

C_PAD = 9216                 # next multiple of 8*128 above C
C_SH = C_PAD // N_CORES      # 1152 classes per core
KCH = D // P                 # 4 contraction chunks
JCH = C_SH // P              # 9 class chunks per core
LCH = 5                      # l chunks
LT = L // LCH                # 500 columns per matmul (fits one PSUM bank)

F32 = mybir.dt.float32
BF16 = mybir.dt.bfloat16
FP8 = mybir.dt.float8e4
FP8_NP = mybir.dt.np(mybir.dt.float8e4)  # ml_dtypes.float8_e4m3

# s1 path in fp8-e4m3 DoubleRow (2x PE throughput on half the matmuls).
# W1 is scaled by 16 into e4m3's normal range; the exp() compensates with
# scale=1/16. s2 stays bf16 since its error enters the output linearly.
FP8_S1 = False
W1_SCALE = 16.0


def build_nc(b=B, kch=KCH, jch=JCH, lch=LCH, lt=LT):
    """Emit the per-core program. All cores run the same NEFF (SPMD)."""
    nc = bacc.Bacc("TRN2", target_bir_lowering=False, debug=False)

    fp8_s1 = FP8_S1
    w1dt = FP8 if fp8_s1 else BF16
    lt8 = (lt + 15) // 16 * 16  # fp8 rhs middle-dim step must be 16B-aligned

    x = nc.dram_tensor("x", [b, kch, P, lch * lt], F32, kind="ExternalInput")
    w1t = nc.dram_tensor("w1t", [kch, P, jch * P], w1dt, kind="ExternalInput")
    w2t = nc.dram_tensor("w2t", [kch, P, jch * P], BF16, kind="ExternalInput")
    b2s = nc.dram_tensor("b2s", [P, jch], F32, kind="ExternalInput")
    out = nc.dram_tensor("out", [jch, P, b], F32, kind="ExternalOutput")

    Exp = mybir.ActivationFunctionType.Exp
    Tanh = mybir.ActivationFunctionType.Tanh
    mult = mybir.AluOpType.mult
    add = mybir.AluOpType.add
    AX = mybir.AxisListType.X

    with tile.TileContext(nc) as tc:
        with (
            tc.tile_pool(name="wts", bufs=1) as wpool,
            tc.tile_pool(name="xraw", bufs=8) as xpool,
            tc.tile_pool(name="xt", bufs=2 * kch * lch) as xtpool,
            tc.tile_pool(name="ps1", bufs=4, space="PSUM") as ppool1,
            tc.tile_pool(name="ps2", bufs=4, space="PSUM") as ppool2,
            tc.tile_pool(name="etile", bufs=6) as epool,
            tc.tile_pool(name="scratch", bufs=4) as spool,
            tc.tile_pool(name="cols", bufs=6) as cpool,
            tc.tile_pool(name="outp", bufs=1) as opool,
        ):
            # one fast HWDGE queue, ordered by first consumption: the first
            # matmul group (j=0, l=0 of batch 0) needs w1 + the four l=0
            # x chunks, then w2 for its s2 half; everything else follows
            w1sb = wpool.tile([P, kch, jch * P], w1dt)
            w2sb = wpool.tile([P, kch, jch * P], BF16)
            b2sb = wpool.tile([P, jch], F32)
            for k in range(kch):
                nc.sync.dma_start(out=w1sb[:, k], in_=w1t[k])

            out_all = opool.tile([P, jch, b], F32)

            for bi in range(b):
                # load + tanh at (k, l-chunk) granularity, l-major order, so
                # the first matmul group's inputs land as early as possible
                xts = {}
                xt8s = {}
                for l in range(lch):
                    if fp8_s1:
                        xt8_l = xtpool.tile([P, kch, lt8], FP8, tag="xt8")
                        xt8s[l] = xt8_l
                    for k in range(kch):
                        xraw = xpool.tile([P, lt], F32)
                        nc.sync.dma_start(
                            out=xraw, in_=x[bi, k, :, l * lt : (l + 1) * lt]
                        )
                        xt_kl = xtpool.tile([P, lt], BF16, tag="xt")
                        nc.scalar.activation(out=xt_kl, in_=xraw, func=Tanh)
                        xts[(k, l)] = xt_kl
                        if fp8_s1:
                            nc.vector.tensor_copy(xt8s[l][:, k, :lt], xt_kl)
                    if bi == 0 and l == 0:
                        for k in range(kch):
                            nc.sync.dma_start(out=w2sb[:, k], in_=w2t[k])
                        nc.sync.dma_start(out=b2sb, in_=b2s[:])

                for j in range(jch):
                    denom_cols = cpool.tile([P, lch], F32, tag="dcols")
                    numer_cols = cpool.tile([P, lch], F32, tag="ncols")
                    for l in range(lch):
                        s1 = ppool1.tile([P, lt], F32)
                        s2 = ppool2.tile([P, lt], F32)
                        if fp8_s1:
                            for pr in range(kch // 2):
                                nc.tensor.matmul(
                                    s1,
                                    w1sb[:, 2 * pr : 2 * pr + 2, ts(j, P)],
                                    xt8s[l][:, 2 * pr : 2 * pr + 2, :lt],
                                    start=(pr == 0),
                                    stop=(pr == kch // 2 - 1),
                                    perf_mode=mybir.MatmulPerfMode.DoubleRow,
                                )
                        else:
                            for k in range(kch):
                                nc.tensor.matmul(
                                    s1,
                                    w1sb[:, k, ts(j, P)],
                                    xts[(k, l)],
                                    start=(k == 0),
                                    stop=(k == kch - 1),
                                )
                        for k in range(kch):
                            nc.tensor.matmul(
                                s2,
                                w2sb[:, k, ts(j, P)],
                                xts[(k, l)],
                                start=(k == 0),
                                stop=(k == kch - 1),
                            )
                        e = epool.tile([P, lt], F32)
                        nc.scalar.activation(
                            out=e, in_=s1, func=Exp,
                            scale=(1.0 / W1_SCALE) if fp8_s1 else 1.0,
                            accum_out=denom_cols[:, l : l + 1],
                        )
                        prod = spool.tile([P, lt], F32)
                        # numer partial = sum_l E * s2 (tensor_tensor_reduce
                        # doesn't execute on this runtime; STT with accum_out
                        # is the same single DVE pass)
                        nc.vector.scalar_tensor_tensor(
                            out=prod, in0=e, scalar=1.0, in1=s2,
                            op0=mult, op1=mult,
                            accum_out=numer_cols[:, l : l + 1],
                        )
                    denom = cpool.tile([P, 1], F32, tag="dsum")
                    numer = cpool.tile([P, 1], F32, tag="nsum")
                    recip = cpool.tile([P, 1], F32, tag="rsum")
                    nc.vector.reduce_sum(denom, denom_cols, axis=AX)
                    nc.vector.reduce_sum(numer, numer_cols, axis=AX)
                    nc.vector.reciprocal(recip, denom)
                    # out = numer * (1/denom) + b2
                    nc.vector.scalar_tensor_tensor(
                        out=out_all[:, j, bi : bi + 1],
                        in0=numer, scalar=recip, in1=b2sb[:, j : j + 1],
                        op0=mult, op1=add,
                    )
                    if bi == b - 1:
                        nc.sync.dma_start(out=out[j], in_=out_all[:, j])

    nc.compile()
    return nc


_NC_CACHE = {}


def _get_nc():
    if "nc" not in _NC_CACHE:
        _NC_CACHE["nc"] = build_nc()
    return _NC_CACHE["nc"]


def make_in_maps(x, W1, W2, b2):
    """Host-side shard prep: pad C, pre-transpose weights, cast to bf16."""
    x = np.ascontiguousarray(np.asarray(x, dtype=np.float32)).reshape(B, KCH, P, L)

    def prep_w(W):
        Wp = np.zeros((C_PAD, D), dtype=np.float32)
        Wp[:C] = np.asarray(W, dtype=np.float32)
        return Wp

    W1p, W2p = prep_w(W1), prep_w(W2)
    b2p = np.zeros((C_PAD,), dtype=np.float32)
    b2p[:C] = np.asarray(b2, dtype=np.float32)

    in_maps = []
    for i in range(N_CORES):
        sl = slice(i * C_SH, (i + 1) * C_SH)
        w1t = np.ascontiguousarray(W1p[sl].T).reshape(KCH, P, C_SH)
        w2t = np.ascontiguousarray(W2p[sl].T).reshape(KCH, P, C_SH)
        b2s = np.ascontiguousarray(b2p[sl].reshape(JCH, P).T)
        if FP8_S1:
            w1c = (w1t * W1_SCALE).astype(FP8_NP)
        else:
            w1c = w1t.astype(ml_dtypes.bfloat16)
        in_maps.append(
            {
                "x": x,
                "w1t": w1c,
                "w2t": w2t.astype(ml_dtypes.bfloat16),
                "b2s": b2s,
            }
        )
    return in_maps


def gather_out(results):
    """results: list (per core) of {'out': [JCH, P, B]} -> full [B, C]."""
    parts = [
        np.transpose(np.asarray(r["out"], dtype=np.float32), (2, 0, 1)).reshape(B, C_SH)
        for r in results
    ]
    return np.concatenate(parts, axis=1)[:, :C]


def kernel(x, W1, W2, b2):
    nc = _get_nc()
    in_maps = make_in_maps(x, W1, W2, b2)
    res = run_bass_kernel_spmd(nc, in_maps, list(range(N_CORES)))
    return gather_out(res.results)
